# revision 16
# baseline (speedup 1.0000x reference)
"""v5: butterfly kernel, two matmul passes + coarse DMA partition-shuffle.

Factor B = Bh @ Bl (increasing stride, 10 stages):
  Bl = stages 0..6  — block-diagonal over 8 contiguous 128-position blocks (w).
  Bh = stages 7..9  — per r in [0,128): an 8x8 matrix H_r over w; identity in r.

Layouts (pos = 128*w + r, r = 16*j + rl), per 1024-batch super-tile:
  xh[r, s, w, b]  = x[s*1024 + b, 128w + r]   (host pre-transpose, bf16;
                    per-super 16KB/partition contiguous -> 128-desc DMAs)
  pass1:  per 512-batch half: 8 matmuls [128x128] x [128,512] -> psum bank,
          evicted (Scalar/Vector alternating) to U[r, w, b] bf16
  shuffle: T[8rl + w, j, b] = U[16j + rl, w, b] - 8 SBUF->SBUF DMAs per super
          (out partition order (rl, w) is natural 0..127; in is a plain
          16-partition slice, 16KB contiguous lines)
  pass2:  out[b, 128w_o + 16j + rl] = sum_{w_i} H_{16j+rl}[w_o, w_i] T[8rl+w_i, j, b]
          per 128-batch chunk ch: 8 matmuls lhsT=T[:, j, chunk], rhs=D_j
          D_j[8rl + w_i, 16w_o + rl'] = delta_{rl,rl'} H_{16j+rl}[w_o, w_i]
  evict:  contiguous [128,1024] copies into ot[p, ch, q] (psum-native column
          order (j, w_o, rl)); one 128-desc DMA per super stores device rows
          d = 8p + ch.
  host:   un-permutes rows (d = 8p+ch -> 128ch+p) and columns
          (c = 128j+16w+rl -> pos = 128w+16j+rl), upconverts bf16->fp32 and
          adds the bias - one O(out) pass that replaces the plain cast.
"""

import os
import sys
import numpy as np

for _p in ("/opt/trn_rl_repo", os.path.expanduser("~/.axon_site/_ro/trn_rl_repo")):
    if os.path.isdir(_p) and _p not in sys.path:
        sys.path.insert(0, _p)

import concourse.bass as bass
import concourse.bacc as bacc
import concourse.mybir as mybir
from concourse import tile
from concourse.bass_utils import run_bass_kernel_spmd

import ml_dtypes

N_CORES = 8
BATCH = 32768
N = 1024
BC = BATCH // N_CORES   # 4096 rows per core
SUP = 1024              # super-tile (shuffle/store granularity)
NSUP = BC // SUP        # 4
BT1 = 512               # pass-1 matmul width

_last_exec_time_ns = None
_nc_cache = None


def _apply_stages(m: np.ndarray, twiddle: np.ndarray, idxs) -> np.ndarray:
    """Apply butterfly stages `idxs` to the rows of m (batch of vectors)."""
    n = N
    for idx in idxs:
        s = 1 << idx
        g = n // (2 * s)
        t = twiddle[0, 0, idx].astype(np.float64).reshape(g, s, 2, 2)
        xr = m.reshape(-1, g, 2, s)
        m = np.einsum("grij,bgjr->bgir", t, xr).reshape(-1, n)
    return m


def _host_weights(twiddle: np.ndarray):
    eye = np.eye(N, dtype=np.float64)
    blt = _apply_stages(eye, twiddle, range(7))        # blt[k, p] = Bl[p, k]
    bht = _apply_stages(eye, twiddle, range(7, 10))    # bht[k, p] = Bh[p, k]

    # pass-1 lhsT: blw[k, w, m] = Bl[128w + m, 128w + k]
    blw = np.zeros((128, 8, 128), dtype=np.float64)
    for w in range(8):
        blw[:, w, :] = blt[128 * w:128 * (w + 1), 128 * w:128 * (w + 1)]

    # pass-2 moving operand: dds[p', j, q], p' = 8 rl + w_i, q = 16 w_o + rl'
    dds = np.zeros((128, 8, 128), dtype=np.float64)
    wi = np.arange(8)
    wo = np.arange(8)
    rl = np.arange(16)
    for j in range(8):
        pos_i = (128 * wi[None, :] + 16 * j + rl[:, None]).ravel()  # (rl, w_i)
        pos_o = (128 * wo[:, None] + 16 * j + rl[None, :]).ravel()  # (w_o, rl')
        sub = bht[np.ix_(pos_i, pos_o)]  # [128, 128]
        row_rl = np.arange(128) // 8
        col_rl = np.arange(128) % 16
        mask = row_rl[:, None] == col_rl[None, :]
        dds[:, j, :] = np.where(mask, sub, 0.0)

    return blw, dds


def _build_nc():
    nc = bacc.Bacc("TRN2", target_bir_lowering=False)
    xtb = nc.dram_tensor("xtb", [128, NSUP, 8, SUP], mybir.dt.bfloat16,
                         kind="ExternalInput")
    bl = nc.dram_tensor("bl", [128, 8, 128], mybir.dt.bfloat16, kind="ExternalInput")
    dd = nc.dram_tensor("dd", [128, 8, 128], mybir.dt.bfloat16, kind="ExternalInput")
    out = nc.dram_tensor("out", [BC, N], mybir.dt.bfloat16, kind="ExternalOutput")

    with tile.TileContext(nc) as tc:
        with (
            tc.tile_pool(name="const", bufs=1) as cpool,
            tc.tile_pool(name="u", bufs=2) as u_pool,
            tc.tile_pool(name="t", bufs=2) as t_pool,
            tc.tile_pool(name="ot", bufs=2) as ot_pool,
            tc.tile_pool(name="ps1", bufs=2, space="PSUM") as ps1_pool,
            tc.tile_pool(name="ps2", bufs=2, space="PSUM") as ps2_pool,
        ):
            bls = cpool.tile([128, 8, 128], mybir.dt.bfloat16)
            nc.sync.dma_start(out=bls[:], in_=bl[:])
            dtl = cpool.tile([128, 8, 128], mybir.dt.bfloat16)
            nc.sync.dma_start(out=dtl[:], in_=dd[:])

            xall = cpool.tile([128, NSUP, 8, SUP], mybir.dt.bfloat16)
            for s in range(NSUP):
                nc.sync.dma_start(out=xall[:, s], in_=xtb[:, s])

            ev = [0]  # alternating eviction engine

            def evict(out_ap, in_ap):
                eng = nc.vector.tensor_copy if ev[0] % 2 == 0 else nc.scalar.copy
                ev[0] += 1
                eng(out_ap, in_ap)

            def p1_units(s):
                """8 generators: one (2 matmuls + 1 two-bank evict) per (t, w-pair)."""
                ut = u_pool.tile([128, 8, SUP], mybir.dt.bfloat16)

                def unit(t, i):
                    tsl = slice(t * BT1, (t + 1) * BT1)
                    ps = ps1_pool.tile([128, 2 * BT1], mybir.dt.float32)
                    for k in range(2):
                        nc.tensor.matmul(
                            ps[:, k * BT1:(k + 1) * BT1],
                            bls[:, 2 * i + k, :],
                            xall[:, s, 2 * i + k, tsl],
                            start=True,
                            stop=True,
                        )
                    evict(
                        ut[:, 2 * i:2 * i + 2, tsl],
                        ps[:].rearrange("p (k b) -> p k b", k=2),
                    )

                units = [
                    (lambda t=t, i=i: unit(t, i))
                    for t in range(SUP // BT1)
                    for i in range(4)
                ]
                return ut, units

            def shuffle(s, ut):
                # shuffle: T[8rl + w, j, b] = U[16j + rl, w, b]
                tt = t_pool.tile([128, 8, SUP], mybir.dt.bfloat16)
                for j in range(8):
                    deng = (nc.sync, nc.scalar)[j % 2]
                    deng.dma_start(
                        out=tt[:, j, :],
                        in_=ut[16 * j:16 * (j + 1), :, :],
                    )
                return tt

            def p2_units(s, tt):
                """8 full-chunk units + final store; device rows d = 8p + ch."""
                ot = ot_pool.tile([128, 8, N], mybir.dt.bfloat16)

                def unit(ch):
                    ps2 = ps2_pool.tile([128, N], mybir.dt.float32)
                    for j in range(8):
                        nc.tensor.matmul(
                            ps2[:, 128 * j:128 * (j + 1)],
                            tt[:, j, 128 * ch:128 * (ch + 1)],
                            dtl[:, j, :],
                            start=True,
                            stop=True,
                        )
                    evict(ot[:, ch, :], ps2[:])
                    if ch == 7:
                        nc.sync.dma_start(
                            out=out[s * SUP:(s + 1) * SUP, :].rearrange(
                                "(p ch) q -> p ch q", p=128, ch=8
                            ),
                            in_=ot[:],
                        )

                return [(lambda ch=ch: unit(ch)) for ch in range(8)]

            # software pipeline with unit-level interleave: pass1(s) units are
            # zipped with pass2(s-1) units so every engine queue alternates
            # between the two phases instead of processing them as waves
            prev_p2 = []
            for s in range(NSUP):
                ut, p1u = p1_units(s)
                for a, b in zip(p1u, prev_p2 + [None] * (len(p1u) - len(prev_p2))):
                    a()
                    if b is not None:
                        b()
                tt = shuffle(s, ut)
                prev_p2 = p2_units(s, tt)
            for b in prev_p2:
                b()

    nc.compile()
    return nc


_COL_SRC = None


def _col_src():
    # natural pos = 128w + 16j + rl  <-  device col c = 128j + 16w + rl
    global _COL_SRC
    if _COL_SRC is None:
        pos = np.arange(N)
        w = pos // 128
        j = (pos % 128) // 16
        rl = pos % 16
        _COL_SRC = (128 * j + 16 * w + rl).astype(np.int64)
    return _COL_SRC


def kernel(x: np.ndarray, twiddle: np.ndarray, bias: np.ndarray) -> np.ndarray:
    global _last_exec_time_ns, _nc_cache

    blw, dds = _host_weights(np.asarray(twiddle))
    bl_host = np.ascontiguousarray(blw.astype(ml_dtypes.bfloat16))
    dd_host = np.ascontiguousarray(dds.astype(ml_dtypes.bfloat16))

    x = np.ascontiguousarray(x, dtype=np.float32)
    xb = x.astype(ml_dtypes.bfloat16)
    # xh[core, r, s, w, b] = x[core, s*1024 + b, 128w + r]
    xtb_all = np.ascontiguousarray(
        xb.reshape(N_CORES, NSUP, SUP, 8, 128).transpose(0, 4, 1, 3, 2)
    )

    if _nc_cache is None:
        _nc_cache = _build_nc()
    nc = _nc_cache

    in_maps = [
        {"xtb": xtb_all[i], "bl": bl_host, "dd": dd_host}
        for i in range(N_CORES)
    ]

    trace = bool(int(os.environ.get("BUTTERFLY_TRACE", "0")))
    res = run_bass_kernel_spmd(
        nc,
        in_maps,
        core_ids=list(range(N_CORES)),
        trace=trace,
    )
    _last_exec_time_ns = res.exec_time_ns

    bias32 = np.asarray(bias, dtype=np.float32)[None, :]
    col_src = _col_src()
    outs = []
    for i in range(N_CORES):
        dev = np.asarray(res.results[i]["out"])  # [BC, N] bf16, rows d = 8p+ch
        dev = dev.reshape(NSUP, 128, 8, N).transpose(0, 2, 1, 3).reshape(BC, N)
        outs.append(dev[:, col_src].astype(np.float32) + bias32)
    return np.concatenate(outs, axis=0)


# revision 17
# speedup vs baseline: 1.0411x; 1.0411x over previous
"""v5: butterfly kernel, two matmul passes + coarse DMA partition-shuffle.

Factor B = Bh @ Bl (increasing stride, 10 stages):
  Bl = stages 0..6  — block-diagonal over 8 contiguous 128-position blocks (w).
  Bh = stages 7..9  — per r in [0,128): an 8x8 matrix H_r over w; identity in r.

Layouts (pos = 128*w + r, r = 16*j + rl), per 1024-batch super-tile:
  xh[r, s, w, b]  = x[s*1024 + b, 128w + r]   (host pre-transpose, bf16;
                    per-super 16KB/partition contiguous -> 128-desc DMAs)
  pass1:  per 512-batch half: 8 matmuls [128x128] x [128,512] -> psum bank,
          evicted (Scalar/Vector alternating) to U[r, w, b] bf16
  shuffle: T[8rl + w, j, b] = U[16j + rl, w, b] - 8 SBUF->SBUF DMAs per super
          (out partition order (rl, w) is natural 0..127; in is a plain
          16-partition slice, 16KB contiguous lines)
  pass2:  out[b, 128w_o + 16j + rl] = sum_{w_i} H_{16j+rl}[w_o, w_i] T[8rl+w_i, j, b]
          per 128-batch chunk ch: 8 matmuls lhsT=T[:, j, chunk], rhs=D_j
          D_j[8rl + w_i, 16w_o + rl'] = delta_{rl,rl'} H_{16j+rl}[w_o, w_i]
  evict:  contiguous [128,1024] copies into ot[p, ch, q] (psum-native column
          order (j, w_o, rl)); one 128-desc DMA per super stores device rows
          d = 8p + ch.
  host:   un-permutes rows (d = 8p+ch -> 128ch+p) and columns
          (c = 128j+16w+rl -> pos = 128w+16j+rl), upconverts bf16->fp32 and
          adds the bias - one O(out) pass that replaces the plain cast.
"""

import os
import sys
import numpy as np

for _p in ("/opt/trn_rl_repo", os.path.expanduser("~/.axon_site/_ro/trn_rl_repo")):
    if os.path.isdir(_p) and _p not in sys.path:
        sys.path.insert(0, _p)

import concourse.bass as bass
import concourse.bacc as bacc
import concourse.mybir as mybir
from concourse import tile
from concourse.bass_utils import run_bass_kernel_spmd

import ml_dtypes

N_CORES = 8
BATCH = 32768
N = 1024
BC = BATCH // N_CORES   # 4096 rows per core
SUP = 1024              # super-tile (shuffle/store granularity)
NSUP = BC // SUP        # 4
BT1 = 512               # pass-1 matmul width

_last_exec_time_ns = None
_nc_cache = None


def _apply_stages(m: np.ndarray, twiddle: np.ndarray, idxs) -> np.ndarray:
    """Apply butterfly stages `idxs` to the rows of m (batch of vectors)."""
    n = N
    for idx in idxs:
        s = 1 << idx
        g = n // (2 * s)
        t = twiddle[0, 0, idx].astype(np.float64).reshape(g, s, 2, 2)
        xr = m.reshape(-1, g, 2, s)
        m = np.einsum("grij,bgjr->bgir", t, xr).reshape(-1, n)
    return m


def _host_weights(twiddle: np.ndarray):
    eye = np.eye(N, dtype=np.float64)
    blt = _apply_stages(eye, twiddle, range(7))        # blt[k, p] = Bl[p, k]
    bht = _apply_stages(eye, twiddle, range(7, 10))    # bht[k, p] = Bh[p, k]

    # pass-1 lhsT: blw[k, w, m] = Bl[128w + m, 128w + k]
    blw = np.zeros((128, 8, 128), dtype=np.float64)
    for w in range(8):
        blw[:, w, :] = blt[128 * w:128 * (w + 1), 128 * w:128 * (w + 1)]

    # pass-2 moving operand: dds[p', j, q], p' = 8 rl + w_i, q = 16 w_o + rl'
    dds = np.zeros((128, 8, 128), dtype=np.float64)
    wi = np.arange(8)
    wo = np.arange(8)
    rl = np.arange(16)
    for j in range(8):
        pos_i = (128 * wi[None, :] + 16 * j + rl[:, None]).ravel()  # (rl, w_i)
        pos_o = (128 * wo[:, None] + 16 * j + rl[None, :]).ravel()  # (w_o, rl')
        sub = bht[np.ix_(pos_i, pos_o)]  # [128, 128]
        row_rl = np.arange(128) // 8
        col_rl = np.arange(128) % 16
        mask = row_rl[:, None] == col_rl[None, :]
        dds[:, j, :] = np.where(mask, sub, 0.0)

    return blw, dds


def _build_nc():
    nc = bacc.Bacc("TRN2", target_bir_lowering=False)
    xtb = nc.dram_tensor("xtb", [128, NSUP, 8, SUP], mybir.dt.bfloat16,
                         kind="ExternalInput")
    bl = nc.dram_tensor("bl", [128, 8, 128], mybir.dt.bfloat16, kind="ExternalInput")
    dd = nc.dram_tensor("dd", [128, 8, 128], mybir.dt.bfloat16, kind="ExternalInput")
    out = nc.dram_tensor("out", [BC, N], mybir.dt.bfloat16, kind="ExternalOutput")

    with tile.TileContext(nc) as tc:
        with (
            tc.tile_pool(name="const", bufs=1) as cpool,
            tc.tile_pool(name="u", bufs=2) as u_pool,
            tc.tile_pool(name="t", bufs=2) as t_pool,
            tc.tile_pool(name="ot", bufs=2) as ot_pool,
            tc.tile_pool(name="ps1", bufs=2, space="PSUM") as ps1_pool,
            tc.tile_pool(name="ps2", bufs=2, space="PSUM") as ps2_pool,
        ):
            bls = cpool.tile([128, 8, 128], mybir.dt.bfloat16)
            nc.sync.dma_start(out=bls[:], in_=bl[:])
            dtl = cpool.tile([128, 8, 128], mybir.dt.bfloat16)
            nc.sync.dma_start(out=dtl[:], in_=dd[:])

            xall = cpool.tile([128, NSUP, 8, SUP], mybir.dt.bfloat16)
            for s in range(NSUP):
                nc.sync.dma_start(out=xall[:, s], in_=xtb[:, s])

            ev = [0]  # alternating eviction engine

            def evict(out_ap, in_ap):
                eng = nc.vector.tensor_copy if ev[0] % 2 == 0 else nc.scalar.copy
                ev[0] += 1
                eng(out_ap, in_ap)

            def p1_units(s):
                """8 generators: one (2 matmuls + 1 two-bank evict) per (t, w-pair)."""
                ut = u_pool.tile([128, 8, SUP], mybir.dt.bfloat16)

                def unit(t, i):
                    tsl = slice(t * BT1, (t + 1) * BT1)
                    ps = ps1_pool.tile([128, 2 * BT1], mybir.dt.float32)
                    for k in range(2):
                        nc.tensor.matmul(
                            ps[:, k * BT1:(k + 1) * BT1],
                            bls[:, 2 * i + k, :],
                            xall[:, s, 2 * i + k, tsl],
                            start=True,
                            stop=True,
                        )
                    evict(
                        ut[:, 2 * i:2 * i + 2, tsl],
                        ps[:].rearrange("p (k b) -> p k b", k=2),
                    )

                units = [
                    (lambda t=t, i=i: unit(t, i))
                    for t in range(SUP // BT1)
                    for i in range(4)
                ]
                return ut, units

            def shuffle(s, ut):
                # shuffle: T[8rl + w, j, b] = U[16j + rl, w, b]
                tt = t_pool.tile([128, 8, SUP], mybir.dt.bfloat16)
                for j in range(8):
                    nc.sync.dma_start(
                        out=tt[:, j, :],
                        in_=ut[16 * j:16 * (j + 1), :, :],
                    )
                return tt

            def p2_units(s, tt):
                """8 full-chunk units + final store; device rows d = 8p + ch."""
                ot = ot_pool.tile([128, 8, N], mybir.dt.bfloat16)

                def unit(ch):
                    ps2 = ps2_pool.tile([128, N], mybir.dt.float32)
                    for j in range(8):
                        nc.tensor.matmul(
                            ps2[:, 128 * j:128 * (j + 1)],
                            tt[:, j, 128 * ch:128 * (ch + 1)],
                            dtl[:, j, :],
                            start=True,
                            stop=True,
                        )
                    evict(ot[:, ch, :], ps2[:])
                    if ch == 7:
                        nc.sync.dma_start(
                            out=out[s * SUP:(s + 1) * SUP, :].rearrange(
                                "(p ch) q -> p ch q", p=128, ch=8
                            ),
                            in_=ot[:],
                        )

                return [(lambda ch=ch: unit(ch)) for ch in range(8)]

            # software pipeline with unit-level interleave: pass1(s) units are
            # zipped with pass2(s-1) units so every engine queue alternates
            # between the two phases instead of processing them as waves
            prev_p2 = []
            for s in range(NSUP):
                ut, p1u = p1_units(s)
                for a, b in zip(p1u, prev_p2 + [None] * (len(p1u) - len(prev_p2))):
                    a()
                    if b is not None:
                        b()
                tt = shuffle(s, ut)
                prev_p2 = p2_units(s, tt)
            for b in prev_p2:
                b()

    nc.compile()
    return nc


_COL_SRC = None


def _col_src():
    # natural pos = 128w + 16j + rl  <-  device col c = 128j + 16w + rl
    global _COL_SRC
    if _COL_SRC is None:
        pos = np.arange(N)
        w = pos // 128
        j = (pos % 128) // 16
        rl = pos % 16
        _COL_SRC = (128 * j + 16 * w + rl).astype(np.int64)
    return _COL_SRC


def kernel(x: np.ndarray, twiddle: np.ndarray, bias: np.ndarray) -> np.ndarray:
    global _last_exec_time_ns, _nc_cache

    blw, dds = _host_weights(np.asarray(twiddle))
    bl_host = np.ascontiguousarray(blw.astype(ml_dtypes.bfloat16))
    dd_host = np.ascontiguousarray(dds.astype(ml_dtypes.bfloat16))

    x = np.ascontiguousarray(x, dtype=np.float32)
    xb = x.astype(ml_dtypes.bfloat16)
    # xh[core, r, s, w, b] = x[core, s*1024 + b, 128w + r]
    xtb_all = np.ascontiguousarray(
        xb.reshape(N_CORES, NSUP, SUP, 8, 128).transpose(0, 4, 1, 3, 2)
    )

    if _nc_cache is None:
        _nc_cache = _build_nc()
    nc = _nc_cache

    in_maps = [
        {"xtb": xtb_all[i], "bl": bl_host, "dd": dd_host}
        for i in range(N_CORES)
    ]

    trace = bool(int(os.environ.get("BUTTERFLY_TRACE", "0")))
    res = run_bass_kernel_spmd(
        nc,
        in_maps,
        core_ids=list(range(N_CORES)),
        trace=trace,
    )
    _last_exec_time_ns = res.exec_time_ns

    bias32 = np.asarray(bias, dtype=np.float32)[None, :]
    col_src = _col_src()
    outs = []
    for i in range(N_CORES):
        dev = np.asarray(res.results[i]["out"])  # [BC, N] bf16, rows d = 8p+ch
        dev = dev.reshape(NSUP, 128, 8, N).transpose(0, 2, 1, 3).reshape(BC, N)
        outs.append(dev[:, col_src].astype(np.float32) + bias32)
    return np.concatenate(outs, axis=0)
